# revision 13
# baseline (speedup 1.0000x reference)
"""Photonic mesh layer (64-mode Reck MZI mesh + Kerr phase) on 8 trn2 cores.

Strategy:
  - The 2016-phase MZI mesh composes (host-side, ~1M flops) into a single
    64x64 complex unitary U; the layer is then out = x @ U.T.
  - The Kerr nonlinearity phase is NONLINEAR_COEFF*|out|^2*1000 ~ 3e-17,
    which is far below float32 resolution -> exp(i*phi) == 1 in f32.
    (Verified: including it changes nothing at the 1e-7 level.)
  - Complex matmul is embedded as a real [B,128] @ [128,128] matmul:
    X = [x_real | x_imag], W[k,2j]=Ur[j,k], W[64+k,2j]=-Ui[j,k],
    W[k,2j+1]=Ui[j,k], W[64+k,2j+1]=Ur[j,k], so the output free dim is
    already (re,im)-interleaved == complex64 memory layout.
  - Host pre-transposes X to feature-major [128, B/8] per core so the
    device does zero on-chip transposes: W stays as the stationary
    operand, batch streams through as the moving operand, and the PSUM
    result [128 out-features, batch] DMAs straight out.
  - Pure data parallel over the batch: core c gets rows [c*B/8, (c+1)*B/8).
"""

import numpy as np

import concourse.bass as bass
import concourse.bacc as bacc
import concourse.mybir as mybir
import concourse.tile as tile
from concourse.bass_utils import run_bass_kernel_spmd

SIZE = 64
N_PHASES = SIZE * (SIZE - 1) // 2  # 2016
LOSS_AMP = np.float32(10.0 ** (-(0.5 * 0.1) / 20.0))
_MODE_IDX = np.array([j for i in range(1, SIZE) for j in range(i)], dtype=np.int32)

N_CORES = 8
B = 262144
B_SHARD = B // N_CORES  # 32768
import os

CHUNK = int(os.environ.get("PHOTONIC_CHUNK", "8192"))  # batch cols per DMA tile
MM_N = 512  # moving free dim per matmul (one PSUM bank of fp32)
MM_DT_NAME = os.environ.get("PHOTONIC_MM_DT", "float32")
XBUFS = int(os.environ.get("PHOTONIC_XBUFS", "2"))
OBUFS = int(os.environ.get("PHOTONIC_OBUFS", "2"))

_NC_CACHE = None


def _build_w(phases: np.ndarray) -> np.ndarray:
    """Compose the Reck mesh into U (complex64), embed as real [128,128] W."""
    phases = np.asarray(phases, dtype=np.float32).reshape(-1)
    assert phases.shape == (N_PHASES,)
    cos = np.cos(phases)
    sin = np.sin(phases)
    U = np.eye(SIZE, dtype=np.complex64)
    for k in range(N_PHASES):
        m = _MODE_IDX[k]
        c = np.complex64(cos[k])
        s = np.complex64(1j * sin[k])
        ra = U[m].copy()
        rb = U[m + 1].copy()
        U[m] = LOSS_AMP * (c * ra + s * rb)
        U[m + 1] = LOSS_AMP * (s * ra + c * rb)
    Ur = np.ascontiguousarray(U.real)
    Ui = np.ascontiguousarray(U.imag)
    W = np.zeros((2 * SIZE, 2 * SIZE), dtype=np.float32)
    W[0:SIZE, 0::2] = Ur.T
    W[SIZE:, 0::2] = -Ui.T
    W[0:SIZE, 1::2] = Ui.T
    W[SIZE:, 1::2] = Ur.T
    return W


def _build_bass() -> bass.Bass:
    # Bacc (not plain Bass): its finalize() runs generate_event_semaphores,
    # which splits multi-semaphore waits to satisfy TRN2's one-wait-per-
    # instruction constraint — plain Bass output fails walrus codegen.
    nc = bacc.Bacc(None, target_bir_lowering=False)
    mm_dt = getattr(mybir.dt, MM_DT_NAME)
    xt = nc.dram_tensor("xt", [128, B_SHARD], mm_dt, kind="ExternalInput")
    w = nc.dram_tensor("w", [128, 128], mm_dt, kind="ExternalInput")
    out = nc.dram_tensor("out", [128, B_SHARD], mybir.dt.float32, kind="ExternalOutput")

    with tile.TileContext(nc) as tc:
        with (
            tc.tile_pool(name="wpool", bufs=1) as wpool,
            tc.tile_pool(name="xpool", bufs=XBUFS) as xpool,
            tc.tile_pool(name="opool", bufs=OBUFS) as opool,
            tc.tile_pool(name="psum", bufs=8, space="PSUM") as psum_pool,
        ):
            w_tile = wpool.tile([128, 128], mm_dt)
            nc.sync.dma_start(w_tile[:], w[:])
            for ci, cstart in enumerate(range(0, B_SHARD, CHUNK)):
                # alternate the two HWDGE rings (SP / ACT) per chunk so
                # input and output descriptor generation run in parallel
                in_ring = nc.sync if ci % 2 == 0 else nc.scalar
                out_ring = nc.scalar if ci % 2 == 0 else nc.sync
                x_tile = xpool.tile([128, CHUNK], mm_dt)
                in_ring.dma_start(x_tile[:], xt[:, cstart : cstart + CHUNK])
                o_tile = opool.tile([128, CHUNK], mybir.dt.float32)
                for s in range(0, CHUNK, MM_N):
                    p = psum_pool.tile([128, MM_N], mybir.dt.float32)
                    nc.tensor.matmul(
                        p[:],
                        w_tile[:],
                        x_tile[:, s : s + MM_N],
                        start=True,
                        stop=True,
                    )
                    nc.vector.tensor_copy(o_tile[:, s : s + MM_N], p[:])
                out_ring.dma_start(out[:, cstart : cstart + CHUNK], o_tile[:])
    nc.finalize()
    return nc


def _get_nc() -> bass.Bass:
    global _NC_CACHE
    if _NC_CACHE is None:
        _NC_CACHE = _build_bass()
    return _NC_CACHE


def kernel(x_real: np.ndarray, x_imag: np.ndarray, phases: np.ndarray, **_kw):
    assert x_real.shape == (B, SIZE) and x_imag.shape == (B, SIZE)
    W = _build_w(phases)

    in_maps = []
    for c in range(N_CORES):
        sl = slice(c * B_SHARD, (c + 1) * B_SHARD)
        xt = np.empty((128, B_SHARD), dtype=np.float32)
        xt[:SIZE] = x_real[sl].T
        xt[SIZE:] = x_imag[sl].T
        in_maps.append({"xt": xt, "w": W})

    nc = _get_nc()
    res = run_bass_kernel_spmd(nc, in_maps, core_ids=list(range(N_CORES)))

    out_f32 = np.empty((B, 2 * SIZE), dtype=np.float32)
    for c in range(N_CORES):
        sl = slice(c * B_SHARD, (c + 1) * B_SHARD)
        out_f32[sl] = res.results[c]["out"].T
    return out_f32.view(np.complex64)


# revision 18
# speedup vs baseline: 1.0586x; 1.0586x over previous
"""Photonic mesh layer (64-mode Reck MZI mesh + Kerr phase) on 8 trn2 cores.

Strategy:
  - The 2016-phase MZI mesh composes (host-side, ~1M flops) into a single
    64x64 complex unitary U; the layer is then out = x @ U.T.
  - The Kerr nonlinearity phase is NONLINEAR_COEFF*|out|^2*1000 ~ 3e-17,
    which is far below float32 resolution -> exp(i*phi) == 1 in f32.
    (Verified: including it changes nothing at the 1e-7 level.)
  - Complex matmul is embedded as a real [B,128] @ [128,128] matmul:
    X = [x_real | x_imag], W[k,2j]=Ur[j,k], W[64+k,2j]=-Ui[j,k],
    W[k,2j+1]=Ui[j,k], W[64+k,2j+1]=Ur[j,k], so the output free dim is
    already (re,im)-interleaved == complex64 memory layout.
  - Host pre-transposes X to feature-major [128, B/8] per core so the
    device does zero on-chip transposes: W stays as the stationary
    operand, batch streams through as the moving operand, and the PSUM
    result [128 out-features, batch] DMAs straight out.
  - Pure data parallel over the batch: core c gets rows [c*B/8, (c+1)*B/8).
"""

import numpy as np

import concourse.bass as bass
import concourse.bacc as bacc
import concourse.mybir as mybir
import concourse.tile as tile
from concourse.bass_utils import run_bass_kernel_spmd

SIZE = 64
N_PHASES = SIZE * (SIZE - 1) // 2  # 2016
LOSS_AMP = np.float32(10.0 ** (-(0.5 * 0.1) / 20.0))
_MODE_IDX = np.array([j for i in range(1, SIZE) for j in range(i)], dtype=np.int32)

N_CORES = 8
B = 262144
B_SHARD = B // N_CORES  # 32768
import os

CHUNK = int(os.environ.get("PHOTONIC_CHUNK", "8192"))  # batch cols per DMA tile
MM_N = 512  # moving free dim per matmul (one PSUM bank of fp32)
MM_DT_NAME = os.environ.get("PHOTONIC_MM_DT", "float32")
XBUFS = int(os.environ.get("PHOTONIC_XBUFS", "2"))
OBUFS = int(os.environ.get("PHOTONIC_OBUFS", "2"))

_NC_CACHE = None


def _build_w(phases: np.ndarray) -> np.ndarray:
    """Compose the Reck mesh into U (complex64), embed as real [128,128] W."""
    phases = np.asarray(phases, dtype=np.float32).reshape(-1)
    assert phases.shape == (N_PHASES,)
    cos = np.cos(phases)
    sin = np.sin(phases)
    U = np.eye(SIZE, dtype=np.complex64)
    for k in range(N_PHASES):
        m = _MODE_IDX[k]
        c = np.complex64(cos[k])
        s = np.complex64(1j * sin[k])
        ra = U[m].copy()
        rb = U[m + 1].copy()
        U[m] = LOSS_AMP * (c * ra + s * rb)
        U[m + 1] = LOSS_AMP * (s * ra + c * rb)
    Ur = np.ascontiguousarray(U.real)
    Ui = np.ascontiguousarray(U.imag)
    W = np.zeros((2 * SIZE, 2 * SIZE), dtype=np.float32)
    W[0:SIZE, 0::2] = Ur.T
    W[SIZE:, 0::2] = -Ui.T
    W[0:SIZE, 1::2] = Ui.T
    W[SIZE:, 1::2] = Ur.T
    return W


def _build_bass() -> bass.Bass:
    # Bacc (not plain Bass): its finalize() runs generate_event_semaphores,
    # which splits multi-semaphore waits to satisfy TRN2's one-wait-per-
    # instruction constraint — plain Bass output fails walrus codegen.
    nc = bacc.Bacc(None, target_bir_lowering=False)
    mm_dt = getattr(mybir.dt, MM_DT_NAME)
    n_chunks = B_SHARD // CHUNK
    # chunk-major layout: each [128, CHUNK] tile is one contiguous HBM block
    xt = nc.dram_tensor("xt", [n_chunks, 128, CHUNK], mm_dt, kind="ExternalInput")
    w = nc.dram_tensor("w", [128, 128], mm_dt, kind="ExternalInput")
    out = nc.dram_tensor(
        "out", [n_chunks, 128, CHUNK], mybir.dt.float32, kind="ExternalOutput"
    )

    with tile.TileContext(nc) as tc:
        with (
            tc.tile_pool(name="wpool", bufs=1) as wpool,
            tc.tile_pool(name="xpool", bufs=XBUFS) as xpool,
            tc.tile_pool(name="opool", bufs=OBUFS) as opool,
            tc.tile_pool(name="psum", bufs=8, space="PSUM") as psum_pool,
        ):
            w_tile = wpool.tile([128, 128], mm_dt)
            nc.sync.dma_start(w_tile[:], w[:])
            for ci in range(n_chunks):
                # inputs on the SP HWDGE ring, outputs on the ACT ring so
                # the two directions' descriptor generation runs in parallel
                x_tile = xpool.tile([128, CHUNK], mm_dt)
                nc.sync.dma_start(x_tile[:], xt[ci])
                o_tile = opool.tile([128, CHUNK], mybir.dt.float32)
                for s in range(0, CHUNK, MM_N):
                    p = psum_pool.tile([128, MM_N], mybir.dt.float32)
                    nc.tensor.matmul(
                        p[:],
                        w_tile[:],
                        x_tile[:, s : s + MM_N],
                        start=True,
                        stop=True,
                    )
                    nc.vector.tensor_copy(o_tile[:, s : s + MM_N], p[:])
                nc.scalar.dma_start(out[ci], o_tile[:])
    nc.finalize()
    return nc


def _get_nc() -> bass.Bass:
    global _NC_CACHE
    if _NC_CACHE is None:
        _NC_CACHE = _build_bass()
    return _NC_CACHE


def make_in_maps(x_real: np.ndarray, x_imag: np.ndarray, phases: np.ndarray):
    """Shard + lay out the inputs for the 8 cores (chunk-major, feature-first)."""
    W = _build_w(phases)
    n_chunks = B_SHARD // CHUNK
    in_maps = []
    for c in range(N_CORES):
        sl = slice(c * B_SHARD, (c + 1) * B_SHARD)
        xt = np.empty((128, B_SHARD), dtype=np.float32)
        xt[:SIZE] = x_real[sl].T
        xt[SIZE:] = x_imag[sl].T
        xt3 = np.ascontiguousarray(
            xt.reshape(128, n_chunks, CHUNK).transpose(1, 0, 2)
        )
        in_maps.append({"xt": xt3, "w": W})
    return in_maps


def assemble_out(results) -> np.ndarray:
    """[n_chunks,128,CHUNK] per-core device outputs -> full [B,64] complex64."""
    out_f32 = np.empty((B, 2 * SIZE), dtype=np.float32)
    for c in range(N_CORES):
        o3 = results[c]["out"]  # [n_chunks, 128, CHUNK]
        sl = slice(c * B_SHARD, (c + 1) * B_SHARD)
        out_f32[sl] = o3.transpose(0, 2, 1).reshape(B_SHARD, 2 * SIZE)
    return out_f32.view(np.complex64)


def kernel(x_real: np.ndarray, x_imag: np.ndarray, phases: np.ndarray, **_kw):
    assert x_real.shape == (B, SIZE) and x_imag.shape == (B, SIZE)
    in_maps = make_in_maps(x_real, x_imag, phases)
    nc = _get_nc()
    res = run_bass_kernel_spmd(nc, in_maps, core_ids=list(range(N_CORES)))
    return assemble_out(res.results)
